# revision 15
# baseline (speedup 1.0000x reference)
"""Distributed 2-hop GCN (scatter-mean propagation) for 8 Trainium2 NeuronCores.

Math: h0 = x @ W.T + b; two hops of h <- segment_mean(h[dst], src) over
edges with self loops (bias folded into h0 — mean-propagation is linear).

Per-core scheme (targets sharded by node id; ~400k edges + 12.5k self
loops per core, host-packed into tiles of 128 edge slots x <=8 targets,
64 tiles per chunk):
  * h0 = x @ W.T + b on PE from a host-transposed bf16 x shard; bias via a
    rank-1 accumulate matmul; result cast to bf16 and AllGathered.
  * per chunk ONE indirect DMA gathers all 8192 source rows (bf16, 128B
    descriptors) from the allgathered table into a [128, 64*64] tile.
  * per tile one PE matmul  msg^T @ sel -> PSUM bank [64 feats, 8 slots];
    64 tiles fill a [64, 512] bank, PE-transposed back to slot-major.
  * ACT copies each transposed tile to SBUF scaled by 1/deg (per-partition
    scalar); ONE indirect scatter per chunk writes the 512 slot rows to
    their target rows in the next node table (dummy slots dropped via
    bounds_check; every real target owns exactly one slot thanks to the
    self-loop edge).
  * AllGather (bf16) between hops; final hop scatters fp32 to the output.
"""

import numpy as np

N = 100000
NCORES = 8
SHARD = N // NCORES           # 12500
GROUPS = 98                   # 98 * 128 = 12544
SHARD_PAD = GROUPS * 128      # 12544
F = 64                        # output features
IN = 128                      # input features
P = 128                       # edge slots per tile
K = 8                         # target slots per tile
TB = 64                       # tiles per chunk (TB*K = 512 slots/chunk)
NUM_LAYERS = 2
NQ = 4                        # SWDGE queues for indirect DMAs
DUMMY = SHARD                 # unused slots scatter 0 to a padding row


# ----------------------------------------------------------------------------
# host-side preprocessing (indices only — no tensor math)
# ----------------------------------------------------------------------------

def _pack_core(tgt_c, src_c):
    """Edges with scatter-target tgt_c (one core's range), gather-source
    src_c.  Greedy big/small pack into tiles of P edge slots / K targets.
    Returns gsrc [T,P] int64, seg [T,P] f32 (slot id, K = unused),
    slot_tgt [T*K] int64 (target node per slot, -1 dummy)."""
    order = np.argsort(tgt_c, kind="stable")
    s = tgt_c[order]
    d = src_c[order]
    tgt_ids, seg_starts = np.unique(s, return_index=True)
    seg_ends = np.append(seg_starts[1:], len(s))
    degs = seg_ends - seg_starts
    assert degs.max() <= P, f"target degree {degs.max()} exceeds tile size"
    bydeg = np.argsort(degs, kind="stable")
    lo, hi = 0, len(bydeg) - 1
    tiles_g, tiles_s, slot_tgt = [], [], []
    cur_g = np.zeros(P, np.int64)
    cur_s = np.full(P, K, np.float32)
    fill = 0
    slots = 0

    def flush():
        nonlocal fill, slots
        slot_tgt.extend([-1] * (K - slots))
        tiles_g.append(cur_g.copy())
        tiles_s.append(cur_s.copy())
        cur_g[:] = 0
        cur_s[:] = K
        fill = 0
        slots = 0

    def put(ti):
        nonlocal fill, slots
        deg = int(degs[ti])
        a = int(seg_starts[ti])
        cur_g[fill:fill + deg] = d[a:a + deg]
        cur_s[fill:fill + deg] = slots
        slot_tgt.append(int(tgt_ids[ti]))
        fill += deg
        slots += 1

    while lo <= hi:
        if fill + int(degs[bydeg[hi]]) <= P and slots < K:
            put(int(bydeg[hi]))
            hi -= 1
        elif fill + int(degs[bydeg[lo]]) <= P and slots < K:
            put(int(bydeg[lo]))
            lo += 1
        else:
            flush()
    if slots or fill:
        flush()
    return (np.stack(tiles_g), np.stack(tiles_s),
            np.array(slot_tgt, np.int64))


def _node_to_row(v):
    """node id -> row in the allgathered (padded-shard) node table."""
    c = v // SHARD
    return c * SHARD_PAD + (v - c * SHARD)


def _prepare(edge_index):
    tgt = np.asarray(edge_index[0], np.int64)   # scatter targets
    src = np.asarray(edge_index[1], np.int64)   # gather sources
    loops = np.arange(N, dtype=np.int64)
    tgt_all = np.concatenate([tgt, loops])
    src_all = np.concatenate([src, loops])
    deg = np.bincount(tgt_all, minlength=N).astype(np.float64)  # >= 1

    packed = []
    for c in range(NCORES):
        base = c * SHARD
        m = (tgt_all >= base) & (tgt_all < base + SHARD)
        packed.append(_pack_core(tgt_all[m], src_all[m]))
    tmax = max(p[0].shape[0] for p in packed)
    nchunk = -(-tmax // TB)
    tpad = nchunk * TB

    cores = []
    for c in range(NCORES):
        g, sgm, st = packed[c]
        t = g.shape[0]
        if t < tpad:
            g = np.concatenate([g, np.zeros((tpad - t, P), np.int64)])
            sgm = np.concatenate([sgm, np.full((tpad - t, P), K, np.float32)])
            st = np.concatenate([st, np.full((tpad - t) * K, -1, np.int64)])
        gi = _node_to_row(g).astype(np.int32)                  # [tpad, P]
        gidx = (gi.reshape(nchunk, TB, P).transpose(2, 0, 1)
                .reshape(P, nchunk * TB).copy())               # [128, nchunk*TB]
        seg = (sgm.reshape(nchunk, TB, P).transpose(2, 0, 1)
               .reshape(P, nchunk * TB).astype(np.float32).copy())
        # scatter index/scale per (chunk k, psum tile q, partition p):
        # partition p of psum tile q holds tile j = q*16 + p//8, slot o = p%8
        stc = st.reshape(nchunk, TB, K)                        # [k, j, o]
        j_of_p = np.arange(P) // K                             # 0..15
        o_of_p = np.arange(P) % K
        tgt_slot = np.empty((nchunk, 4, P), np.int64)          # [k, q, p] -> target
        for q in range(4):
            tgt_slot[:, q, :] = stc[:, q * 16 + j_of_p, o_of_p]
        cidx = np.where(tgt_slot >= 0, tgt_slot - c * SHARD, DUMMY)
        cscale = np.where(tgt_slot >= 0,
                          1.0 / np.maximum(deg[np.clip(tgt_slot, 0, N - 1)], 1.0),
                          0.0)
        # layout [128, nchunk*4]
        cidx = cidx.transpose(2, 0, 1).reshape(P, nchunk * 4).astype(np.int32).copy()
        cscale = cscale.transpose(2, 0, 1).reshape(P, nchunk * 4).astype(np.float32).copy()
        cores.append(dict(gidx=gidx, seg=seg, cidx=cidx, cscale=cscale))
    return cores, nchunk


# ----------------------------------------------------------------------------
# walrus workaround: this compiler build accepts at most ONE sync-wait per
# instruction; move extra waits onto preceding NoOps on the same engine.
# ----------------------------------------------------------------------------

def _split_sync_waits(nc, mybir):
    n = 0
    for f in nc.m.functions:
        for b in f.blocks:
            out = []
            changed = False
            for inst in b.instructions:
                si = inst.sync_info
                waits = list(si.on_wait) if si is not None and si.on_wait else []
                if len(waits) > 1:
                    changed = True
                    for w in waits[:-1]:
                        nop = mybir.InstNoOp(
                            name=f"wsplit_{b.name}_{n}", ins=[], outs=[],
                            engine=inst.engine,
                        )
                        n += 1
                        nop.sync_info = mybir.SyncInfo(on_wait=[w], on_update=[])
                        out.append(nop)
                    si.on_wait = waits[-1:]
                out.append(inst)
            if changed:
                b.instructions = out
    return n


def _patch_bass(bass, mybir):
    if getattr(bass.Bass, "_gcn_wait_patch", False):
        return
    orig = bass.Bass.to_json_bytes

    def patched(self, *a, **k):
        _split_sync_waits(self, mybir)
        return orig(self, *a, **k)

    bass.Bass.to_json_bytes = patched
    bass.Bass._gcn_wait_patch = True


# ----------------------------------------------------------------------------
# bass program
# ----------------------------------------------------------------------------

def _build(nchunk, repeat=1):
    import concourse.bass as bass
    import concourse.mybir as mybir
    from concourse.tile import TileContext

    _patch_bass(bass, mybir)

    dt = mybir.dt
    nc = bass.Bass(num_swdge_queues=NQ)

    xT = nc.dram_tensor("xT", [IN, SHARD_PAD], dt.bfloat16, kind="ExternalInput")
    wt = nc.dram_tensor("wt", [IN, F], dt.bfloat16, kind="ExternalInput")
    brow = nc.dram_tensor("brow", [1, F], dt.bfloat16, kind="ExternalInput")
    id64 = nc.dram_tensor("id64", [64, 64], dt.float32, kind="ExternalInput")
    iota = nc.dram_tensor("iota", [128, TB * K], dt.float32, kind="ExternalInput")
    gidx = nc.dram_tensor("gidx", [128, nchunk * TB], dt.int32, kind="ExternalInput")
    seg = nc.dram_tensor("seg", [128, nchunk * TB], dt.float32, kind="ExternalInput")
    cidx = nc.dram_tensor("cidx", [128, nchunk * 4], dt.int32, kind="ExternalInput")
    cscale = nc.dram_tensor("cscale", [128, nchunk * 4], dt.float32,
                            kind="ExternalInput")
    out = nc.dram_tensor("out", [SHARD_PAD, F], dt.float32, kind="ExternalOutput")

    h_loc = [nc.dram_tensor(f"h{i}_loc", [SHARD_PAD, F], dt.bfloat16)
             for i in range(NUM_LAYERS)]
    h_glob = [nc.dram_tensor(f"h{i}_glob", [NCORES * SHARD_PAD, F], dt.bfloat16,
                             addr_space="Shared")
              for i in range(NUM_LAYERS)]

    with TileContext(nc) as tc:
        import contextlib
        with contextlib.ExitStack() as ctx:
            cpool = ctx.enter_context(tc.tile_pool(name="consts", bufs=1))
            mpool = ctx.enter_context(tc.tile_pool(name="msgs", bufs=24))
            opool = ctx.enter_context(tc.tile_pool(name="outs", bufs=3))
            ppool = ctx.enter_context(tc.tile_pool(name="psum", bufs=2, space="PSUM"))
            qpool = ctx.enter_context(tc.tile_pool(name="psum_t", bufs=4, space="PSUM"))

            _qn = [0]

            def _queue(inst):
                if NQ > 1:
                    q = _qn[0] % NQ
                    _qn[0] += 1
                    if q:
                        inst.ins.queue = f"qPoolDynamic{q}"
                return inst

            # ---- persistent SBUF state ----
            xT_sb = cpool.tile([IN, SHARD_PAD], dt.bfloat16)
            nc.sync.dma_start(out=xT_sb[:], in_=xT[:])
            wt_sb = cpool.tile([IN, F], dt.bfloat16)
            nc.sync.dma_start(out=wt_sb[:], in_=wt[:])
            brow_sb = cpool.tile([1, F], dt.bfloat16)
            nc.sync.dma_start(out=brow_sb[:], in_=brow[:])
            id64_sb = cpool.tile([64, 64], dt.float32)
            nc.sync.dma_start(out=id64_sb[:], in_=id64[:])
            one_sb = cpool.tile([1, 128], dt.bfloat16)
            nc.vector.memset(one_sb[:], 1.0)
            iota_sb = cpool.tile([128, TB * K], dt.float32)
            nc.sync.dma_start(out=iota_sb[:], in_=iota[:])
            gidx_sb = cpool.tile([128, nchunk * TB], dt.int32)
            nc.sync.dma_start(out=gidx_sb[:], in_=gidx[:])
            seg_sb = cpool.tile([128, nchunk * TB], dt.float32)
            nc.sync.dma_start(out=seg_sb[:], in_=seg[:])
            cidx_sb = cpool.tile([128, nchunk * 4], dt.int32)
            nc.sync.dma_start(out=cidx_sb[:], in_=cidx[:])
            cscale_sb = cpool.tile([128, nchunk * 4], dt.float32)
            nc.sync.dma_start(out=cscale_sb[:], in_=cscale[:])
            sel_all = cpool.tile([128, nchunk * TB * K], dt.bfloat16)
            h0_all = cpool.tile([128, GROUPS * F], dt.bfloat16)

            def h0_stage(_r):
                """h0 = x @ W.T + b  -> h0_all (bf16) -> h_loc[0]"""
                for g in range(GROUPS):
                    hp = ppool.tile([128, F], dt.float32, name=f"hp{_r}_{g}",
                                    tag="psum", space="PSUM")
                    nc.tensor.matmul(out=hp[:], lhsT=xT_sb[:, g * 128:(g + 1) * 128],
                                     rhs=wt_sb[:], start=True, stop=False)
                    nc.tensor.matmul(out=hp[:], lhsT=one_sb[:], rhs=brow_sb[:],
                                     start=False, stop=True)
                    nc.scalar.activation(
                        out=h0_all[:, g * F:(g + 1) * F], in_=hp[:],
                        func=mybir.ActivationFunctionType.Copy)
                nc.sync.dma_start(
                    out=h_loc[0][:].rearrange("(g p) f -> p g f", p=128),
                    in_=h0_all[:].rearrange("p (g f) -> p g f", f=F))

            def allgather(i):
                nc.gpsimd.collective_compute(
                    "AllGather",
                    mybir.AluOpType.bypass,
                    ins=[h_loc[i].ap()],
                    outs=[h_glob[i].ap()],
                    replica_groups=[list(range(NCORES))],
                )

            def sel_stage(_r):
                for k in range(nchunk):
                    nc.vector.tensor_tensor(
                        out=sel_all[:, k * TB * K:(k + 1) * TB * K]
                            .rearrange("p (t o) -> p t o", o=K),
                        in0=seg_sb[:, k * TB:(k + 1) * TB]
                            .rearrange("p (t o) -> p t o", o=1)
                            .to_broadcast([128, TB, K]),
                        in1=iota_sb[:].rearrange("p (t o) -> p t o", o=K),
                        op=mybir.AluOpType.is_equal,
                    )

            def hop(i, _r):
                """h_glob[i] -> h_loc[i+1] (bf16) or out (fp32, last)"""
                last = i == NUM_LAYERS - 1
                odt = dt.float32 if last else dt.bfloat16
                dest = out if last else h_loc[i + 1]
                for k in range(nchunk):
                    bank = ppool.tile([64, TB * K], dt.float32,
                                      name=f"bk{i}r{_r}_{k}", tag="bank",
                                      space="PSUM")
                    for j in range(TB):
                        msg = mpool.tile([128, F], dt.bfloat16,
                                         name=f"msg{i}r{_r}_{k}_{j}", tag="msg")
                        t = k * TB + j
                        _queue(nc.gpsimd.indirect_dma_start(
                            out=msg[:], out_offset=None, in_=h_glob[i][:],
                            in_offset=bass.IndirectOffsetOnAxis(
                                ap=gidx_sb[:, t:t + 1], axis=0),
                        ))
                        nc.tensor.matmul(
                            out=bank[:, j * K:(j + 1) * K],
                            lhsT=msg[:],
                            rhs=sel_all[:, t * K:(t + 1) * K],
                            start=True, stop=True,
                        )
                    bsb = opool.tile([64, TB * K], dt.float32,
                                     name=f"bs{i}r{_r}_{k}", tag="bsb")
                    nc.vector.tensor_copy(out=bsb[:], in_=bank[:])
                    outt = opool.tile([128, 4 * F], odt,
                                      name=f"ot{i}r{_r}_{k}", tag="outt")
                    for q in range(4):
                        tps = qpool.tile([128, 64], dt.float32,
                                         name=f"tp{i}r{_r}_{k}_{q}", tag="tps",
                                         space="PSUM")
                        nc.tensor.transpose(out=tps[:],
                                            in_=bsb[:, q * 128:(q + 1) * 128],
                                            identity=id64_sb[:])
                        nc.scalar.activation(
                            out=outt[:, q * F:(q + 1) * F], in_=tps[:],
                            func=mybir.ActivationFunctionType.Copy,
                            scale=cscale_sb[:, k * 4 + q:k * 4 + q + 1])
                        _queue(nc.gpsimd.indirect_dma_start(
                            out=dest[:], out_offset=bass.IndirectOffsetOnAxis(
                                ap=cidx_sb[:, k * 4 + q:k * 4 + q + 1], axis=0),
                            in_=outt[:, q * F:(q + 1) * F], in_offset=None,
                        ))

            for _r in range(repeat):
                h0_stage(_r)
                allgather(0)
                sel_stage(_r)
                hop(0, _r)
                allgather(1)
                hop(1, _r)

    return nc


# ----------------------------------------------------------------------------
# entry point
# ----------------------------------------------------------------------------

def _make_in_maps(x, W, b, cores):
    from ml_dtypes import bfloat16

    x = np.asarray(x, np.float32)
    W = np.asarray(W, np.float32)
    b = np.asarray(b, np.float32)
    iota = np.tile(np.arange(K, dtype=np.float32), (128, TB))
    in_maps = []
    for c in range(NCORES):
        xs = np.zeros((SHARD_PAD, IN), np.float32)
        xs[:SHARD] = x[c * SHARD:(c + 1) * SHARD]
        in_maps.append({
            "xT": np.ascontiguousarray(xs.T).astype(bfloat16),
            "wt": np.ascontiguousarray(W.T).astype(bfloat16),
            "brow": b[None, :].astype(bfloat16),
            "id64": np.eye(64, dtype=np.float32),
            "iota": iota,
            "gidx": cores[c]["gidx"],
            "seg": cores[c]["seg"],
            "cidx": cores[c]["cidx"],
            "cscale": cores[c]["cscale"],
        })
    return in_maps


def kernel(x, W, b, edge_index):
    from concourse import bass_utils

    x = np.asarray(x, np.float32)
    W = np.asarray(W, np.float32)
    b = np.asarray(b, np.float32)
    edge_index = np.asarray(edge_index)

    cores, nchunk = _prepare(edge_index)
    nc = _build(nchunk)
    in_maps = _make_in_maps(x, W, b, cores)

    res = bass_utils.run_bass_kernel_spmd(nc, in_maps, core_ids=list(range(NCORES)))
    outp = np.concatenate([res.results[c]["out"][:SHARD] for c in range(NCORES)],
                          axis=0)
    return outp.astype(np.float32)


if __name__ == "__main__":
    import importlib.util
    spec = importlib.util.spec_from_file_location("refmod", "/root/problem/reference.py")
    ref = importlib.util.module_from_spec(spec)
    spec.loader.exec_module(ref)
    inputs = {k: np.asarray(v) for k, v in ref.setup_inputs().items()}
    got = kernel(**inputs)
    print("kernel output", got.shape, got.dtype)
